# revision 16
# baseline (speedup 1.0000x reference)
"""Trainium2 Bass kernel for nn_MixNode (soft decision tree / MoE routing).

The recursive MixNode tree collapses algebraically:
    out[b] = sum_m C_m(x_b) * leafG[m]
where leafG folds the (input-independent) gamma-softmax products into the
leaf vectors, and C_m = prod of routing probabilities along the root->m
path. With delta = logit0 - logit1 per internal node, the two routing
probs are sigmoid(+-delta), so log C = A @ (-softplus(-+delta)) for a
constant 0/-1 path matrix A. softplus(z) = ln(exp(z) + 1) is computed as
Exp then Ln(x + 1); both +-delta blocks are produced by one doubled
matmul (weights [-Wd; +Wd]) so each chunk needs only one Exp + one Ln.

Per core the 2048-row batch shard runs as 5 chunks [256,512,512,512,256]
-- a small first chunk so compute starts one DMA beat earlier, and a
small last chunk so the serial tail after the final DMA arrival is
short. Pipeline per 256-column half:
    gpsimd DMA x (f32 DRAM -> bf16 SBUF cast in the DMA engines) ->
    PE transpose (bf16, 1 cyc/row; 8 tiles into one PSUM bank) ->
    one DVE 16-bit copy to SBUF -> mm1 D2 = [-Wd;+Wd] @ x^T (bf16,
    fp32 accum) -> ACT exp -> ACT ln1p -> mmA S = A @ sp (fp32r) ->
    ACT exp -> C -> mm2 out^T = leafG^T @ C (fp32r) -> DVE copy ->
    DMA out (scalar HW queue).

bf16 is used only on the x side (x, Wd): delta keeps fp32 PSUM
accumulation so the routing logits carry ~2e-3 relative error; the
log-prob algebra stays fp32r. Overall rel err ~3e-4 vs the 2e-2 gate.
The output is produced output-major [128, 2048] per core so mm2 streams
whole chunks and the store DMA uses 2 KiB descriptors; the host
transposes when gathering. Exp/Ln are pinned to one ACT table set
(single table load).

Queues: x loads alone on the gpsimd software-DGE queue (the only engine
that can cast during DMA), constants then y stores on the scalar HW
queue, nothing on sync. PE warm-up matmuls cover the ~5us from the
framework preamble to the first x arrival and push the HAM duty-cycle
governor toward full clock.

Sharding: pure data parallelism over the batch dim across 8 cores;
the small tree parameters are folded host-side and replicated.
"""

import os
import sys

import numpy as np

for _p in ("/opt/trn_rl_repo", "/root/.axon_site/_ro/trn_rl_repo"):
    if os.path.isdir(_p) and _p not in sys.path:
        sys.path.append(_p)

import concourse.tile as tile
from concourse import bacc, mybir
from concourse.bass_utils import run_bass_kernel_spmd

N_CORES = 8
BATCH, D_IN, D_OUT = 16384, 512, 128
B_CORE = BATCH // N_CORES  # 2048
N_INT, N_ALL = 31, 63
P = 128
KC = 4                      # 128-feature chunks
CHUNKS = [256, 512, 512, 512, 256]  # batch columns per chunk
PKBW = 128 + 256            # bf16 const block: ident | wdT
PKFW = 63 + 1 + 128         # f32r const block: aT | biasN | leafG

F32 = mybir.dt.float32
F32R = mybir.dt.float32r
BF16 = mybir.dt.bfloat16
AF = mybir.ActivationFunctionType

N_WARM = 5  # PE clock-ramp matmuls covering the first x DMA latency


def _emit(nc):
    x_d = nc.dram_tensor("x", [B_CORE, D_IN], F32, kind="ExternalInput")
    pkb_d = nc.dram_tensor("pkb", [P, PKBW], BF16, kind="ExternalInput")
    pkf_d = nc.dram_tensor("pkf", [P, PKFW], F32R, kind="ExternalInput")
    y_d = nc.dram_tensor("y", [D_OUT, B_CORE], F32, kind="ExternalOutput")

    with tile.TileContext(nc) as tc:
        with (
            tc.tile_pool(name="const", bufs=1) as constp,
            tc.tile_pool(name="xin", bufs=5) as xinp,
            tc.tile_pool(name="xtp", bufs=2) as xtpp,
            tc.tile_pool(name="act", bufs=2) as actp,
            tc.tile_pool(name="spc", bufs=2) as spp,
            tc.tile_pool(name="ccp", bufs=2) as ccp,
            tc.tile_pool(name="osbp", bufs=2) as osbp,
            tc.tile_pool(name="tps", bufs=3, space="PSUM") as tpsp,
            tc.tile_pool(name="dps", bufs=2, space="PSUM") as dpsp,
            tc.tile_pool(name="sps", bufs=1, space="PSUM") as spsp,
            tc.tile_pool(name="ops", bufs=1, space="PSUM") as opsp,
            tc.tile_pool(name="wps", bufs=1, space="PSUM") as wpsp,
        ):
            # Prefetch the full x shard: one casting DMA (f32 -> bf16) per
            # 256-row group on the gpsimd software-DGE queue.
            xin_l = []
            off = 0
            for w in CHUNKS:
                xin = xinp.tile([P, w // P, D_IN], BF16, tag="xin")
                for h in range(w // 256):
                    src = x_d[off + h * 256:off + (h + 1) * 256, :]
                    nc.gpsimd.dma_start(
                        xin[:, 2 * h:2 * h + 2, :],
                        src.rearrange("(s p) f -> p s f", p=P))
                xin_l.append(xin)
                off += w

            # Constants on the scalar HW queue. f32r tiles must be
            # *produced* as f32r for the BIR verifier; the ACT bias slice
            # is bitcast back (same bits).
            pkb = constp.tile([P, PKBW], BF16)
            nc.scalar.dma_start(pkb[:], pkb_d[:])
            ident = pkb[:, 0:128]
            wdT = [pkb[:, 128 + 64 * k:128 + 64 * (k + 1)] for k in range(KC)]
            pkf = constp.tile([P, PKFW], F32R)
            nc.scalar.dma_start(pkf[:], pkf_d[:])
            aT = pkf[0:64, 0:63]
            biasN = pkf[0:64, 63:64].bitcast(F32)
            leafG = pkf[0:63, 64:192]

            # PE warm-up: dummy fp32 matmuls on a zeroed scratch tile keep
            # the PE busy (HAM clock-gate ramp) until the first x group
            # lands.
            warm = constp.tile([P, P], F32)
            nc.vector.memset(warm[:], 0.0)
            wps = wpsp.tile([P, 512], F32, tag="wps")
            for w in range(N_WARM):
                nc.tensor.matmul(
                    wps[:, (w % 4) * P:(w % 4 + 1) * P], warm[:], warm[:],
                    start=True, stop=True)

            state = {}

            def emit_head(c):
                # One 256-row half at a time: 8 bf16 transposes into one
                # PSUM bank -> one 16-bit DVE copy -> then mm1 over the
                # whole chunk.
                w = CHUNKS[c]
                xin = xin_l[c]
                xT = xtpp.tile([P, KC, w], BF16, tag="xT")
                dps = dpsp.tile([64, w], F32, tag="dps")
                for h in range(w // 256):
                    tps = tpsp.tile([P, KC, 256], BF16, tag="tps")
                    for k in range(KC):
                        for s2 in range(2):
                            s = 2 * h + s2
                            nc.tensor.matmul(
                                tps[:, k, s2 * P:(s2 + 1) * P],
                                xin[:, s, k * P:(k + 1) * P],
                                ident,
                                is_transpose=True,
                                start=(s2 == 0),
                                stop=(s2 == 1),
                            )
                    nc.vector.tensor_copy(
                        xT[:, :, h * 256:(h + 1) * 256], tps[:])
                for k in range(KC):
                    nc.tensor.matmul(
                        dps[:], wdT[k], xT[:, k, :],
                        start=(k == 0), stop=(k == KC - 1))

                # softplus blocks: t = exp(D2 + bias); sp = ln(t + 1)
                # (pad rows give ln2, nulled by the zero rows of A).
                t = actp.tile([64, w], F32, tag="texp")
                nc.scalar.activation(t[:], dps[:], AF.Exp, bias=biasN)
                sp = spp.tile([64, w], F32R, tag="sp")
                nc.scalar.activation(sp[:], t[:], AF.Ln, bias=1.0)
                state[c] = sp

            def emit_tail(c):
                # S = A @ sp, C = exp(S), out^T = leafG^T @ C, store.
                w = CHUNKS[c]
                off = sum(CHUNKS[:c])
                sp = state.pop(c)
                sps = spsp.tile([N_ALL, w], F32, tag="sps")
                nc.tensor.matmul(sps[:], aT, sp[:], start=True, stop=True)
                cc = ccp.tile([N_ALL, w], F32R, tag="cc")
                nc.scalar.activation(cc[:], sps[:], AF.Exp)
                ops = opsp.tile([P, w], F32, tag="ops")
                nc.tensor.matmul(ops[:], leafG, cc[:], start=True, stop=True)
                osb = osbp.tile([P, w], F32, tag="osb")
                nc.vector.tensor_copy(osb[:], ops[:])
                nc.scalar.dma_start(y_d[:, off:off + w], osb[:])

            # Software-pipelined emission: chunk c+1's transposes+mm1 go
            # to the PE queue before chunk c's mmA/mm2 so the PE never
            # stalls waiting on the ACT exp/ln chain.
            emit_head(0)
            for c in range(1, len(CHUNKS)):
                emit_head(c)
                emit_tail(c - 1)
            emit_tail(len(CHUNKS) - 1)
    return nc


_BUILD_CACHE = {}


def _pin_act_tables(nc):
    """Restrict Exp/Ln to the one table set that holds both, so the
    table-load placement pass emits a single ACT_TABLE_LOAD instead of
    thrashing between per-function sets (~2.7us per switch)."""
    from concourse import hw_specs
    tables = hw_specs.get_activation_tables(nc.m.arch)
    both = "natural_log_exp_and_others"
    if both in tables and AF.Exp in tables[both] and AF.Ln in tables[both]:
        for name, fns in tables.items():
            if name != both:
                fns.discard(AF.Exp)
                fns.discard(AF.Ln)


def build():
    if "nc" not in _BUILD_CACHE:
        nc = bacc.Bacc("TRN2", target_bir_lowering=False, debug=False,
                       num_devices=N_CORES)
        _pin_act_tables(nc)
        _emit(nc)
        nc.compile()
        _BUILD_CACHE["nc"] = nc
    return _BUILD_CACHE["nc"]


def host_prep(W, b, gamma, leaf):
    """Fold the tiny tree parameters into two packed constant blocks."""
    import ml_dtypes
    W = np.asarray(W, np.float32)
    b = np.asarray(b, np.float32)
    gamma = np.asarray(gamma, np.float32)
    leaf = np.asarray(leaf, np.float32)

    Wd = W[:, 0, :] - W[:, 1, :]                      # [31, 512]
    bd = b[:, 0] - b[:, 1]                            # [31]
    e = np.exp(gamma - gamma.max(-1, keepdims=True))
    g = e / e.sum(-1, keepdims=True)                  # [31, 2]

    path = np.zeros(N_ALL, np.float64)
    path[0] = 1.0
    for m in range(1, N_ALL):
        par = (m - 1) // 2
        path[m] = path[par] * g[par, 0]
    G = np.array([path[m] * (g[m, 1] if m < N_INT else 1.0)
                  for m in range(N_ALL)])
    leafG = (G[:, None] * leaf.astype(np.float64)).astype(np.float32)

    # A[row, m] = -1 if the edge lives on the root->m path.
    # Edge (node a, child j) -> row a (j=0) or row 32+a (j=1); rows 31/63 pad.
    A = np.zeros((64, N_ALL), np.float32)
    for m in range(N_ALL):
        node = m
        while node:
            par = (node - 1) // 2
            j = node - 2 * par - 1
            A[par if j == 0 else 32 + par, m] = -1.0
            node = par

    # bf16 block: transpose identity | doubled routing weights
    # (cols 0..30 = -Wd^T, cols 32..62 = +Wd^T per 128-feature chunk).
    pkb = np.zeros((P, PKBW), np.float32)
    pkb[:, 0:128] = np.eye(P, dtype=np.float32)
    wdTfull = np.ascontiguousarray(Wd.T)              # [512, 31]
    for k in range(KC):
        blk = wdTfull[k * P:(k + 1) * P]
        pkb[:, 128 + 64 * k + 0:128 + 64 * k + N_INT] = -blk
        pkb[:, 128 + 64 * k + 32:128 + 64 * k + 32 + N_INT] = blk

    # f32r block: aT | bias | leafG.
    pkf = np.zeros((P, PKFW), np.float32)
    pkf[0:64, 0:63] = A
    pkf[0:N_INT, 63] = -bd
    pkf[32:32 + N_INT, 63] = bd
    pkf[0:N_ALL, 64:192] = leafG
    return {"pkb": pkb.astype(ml_dtypes.bfloat16), "pkf": pkf}


def run(x, W, b, gamma, leaf, **spmd_kwargs):
    x = np.asarray(x, np.float32)
    consts = host_prep(W, b, gamma, leaf)
    shards = x.reshape(N_CORES, B_CORE, D_IN)
    in_maps = [dict(consts, x=np.ascontiguousarray(shards[i]))
               for i in range(N_CORES)]
    nc = build()
    res = run_bass_kernel_spmd(nc, in_maps, list(range(N_CORES)), **spmd_kwargs)
    y = np.concatenate(
        [np.ascontiguousarray(res.results[i]["y"].T) for i in range(N_CORES)],
        axis=0)
    return y, res


def kernel(x, W, b, gamma, leaf):
    y, _ = run(x, W, b, gamma, leaf)
    return y


# revision 18
# speedup vs baseline: 1.1115x; 1.1115x over previous
"""Trainium2 Bass kernel for nn_MixNode (soft decision tree / MoE routing).

The recursive MixNode tree collapses algebraically:
    out[b] = sum_m C_m(x_b) * leafG[m]
where leafG folds the (input-independent) gamma-softmax products into the
leaf vectors, and C_m = prod of routing probabilities along the root->m
path. With delta = logit0 - logit1 per internal node, the two routing
probs are sigmoid(+-delta), so log C = A @ (-softplus(-+delta)) for a
constant 0/-1 path matrix A. softplus(z) = ln(exp(z) + 1) is computed as
Exp then Ln(x + 1); both +-delta blocks are produced by one doubled
matmul (weights [-Wd; +Wd]) so each chunk needs only one Exp + one Ln.

Per core the 2048-row batch shard runs as 5 chunks [256,512,512,512,256]
-- a small first chunk so compute starts one DMA beat earlier, and a
small last chunk so the serial tail after the final DMA arrival is
short. Pipeline per 256-column half:
    gpsimd DMA x (f32 DRAM -> bf16 SBUF cast in the DMA engines) ->
    PE transpose (bf16, 1 cyc/row; 8 tiles into one PSUM bank) ->
    one DVE 16-bit copy to SBUF -> mm1 D2 = [-Wd;+Wd] @ x^T (bf16,
    fp32 accum) -> ACT exp -> ACT ln1p -> mmA S = A @ sp (fp32r) ->
    ACT exp -> C -> mm2 out^T = leafG^T @ C (fp32r) -> DVE copy ->
    DMA out (scalar HW queue).

bf16 is used only on the x side (x, Wd): delta keeps fp32 PSUM
accumulation so the routing logits carry ~2e-3 relative error; the
log-prob algebra stays fp32r. Overall rel err ~3e-4 vs the 2e-2 gate.
The output is produced output-major [128, 2048] per core so mm2 streams
whole chunks and the store DMA uses 2 KiB descriptors; the host
transposes when gathering. Exp/Ln are pinned to one ACT table set
(single table load).

Queues: x loads alone on the gpsimd software-DGE queue (the only engine
that can cast during DMA), constants then y stores on the scalar HW
queue, nothing on sync. PE warm-up matmuls cover the ~5us from the
framework preamble to the first x arrival and push the HAM duty-cycle
governor toward full clock.

Sharding: pure data parallelism over the batch dim across 8 cores;
the small tree parameters are folded host-side and replicated.
"""

import os
import sys

import numpy as np

for _p in ("/opt/trn_rl_repo", "/root/.axon_site/_ro/trn_rl_repo"):
    if os.path.isdir(_p) and _p not in sys.path:
        sys.path.append(_p)

import concourse.tile as tile
from concourse import bacc, mybir
from concourse.bass_utils import run_bass_kernel_spmd

N_CORES = 8
BATCH, D_IN, D_OUT = 16384, 512, 128
B_CORE = BATCH // N_CORES  # 2048
N_INT, N_ALL = 31, 63
P = 128
KC = 4                      # 128-feature chunks
CHUNKS = [256, 512, 512, 512, 256]  # batch columns per chunk
PKBW = 128 + 256            # bf16 const block: ident | wdT
PKFW = 63 + 1 + 128         # f32r const block: aT | biasN | leafG

F32 = mybir.dt.float32
F32R = mybir.dt.float32r
BF16 = mybir.dt.bfloat16
AF = mybir.ActivationFunctionType

N_WARM = int(os.environ.get("MIX_WARM", "5"))  # PE clock-ramp matmuls covering the first x DMA latency


def _emit(nc):
    x_d = nc.dram_tensor("x", [B_CORE, D_IN], F32, kind="ExternalInput")
    pkb_d = nc.dram_tensor("pkb", [P, PKBW], BF16, kind="ExternalInput")
    pkf_d = nc.dram_tensor("pkf", [P, PKFW], F32R, kind="ExternalInput")
    y_d = nc.dram_tensor("y", [D_OUT, B_CORE], F32, kind="ExternalOutput")

    with tile.TileContext(nc) as tc:
        with (
            tc.tile_pool(name="const", bufs=1) as constp,
            tc.tile_pool(name="xin", bufs=5) as xinp,
            tc.tile_pool(name="xtp", bufs=2) as xtpp,
            tc.tile_pool(name="act", bufs=2) as actp,
            tc.tile_pool(name="spc", bufs=2) as spp,
            tc.tile_pool(name="ccp", bufs=2) as ccp,
            tc.tile_pool(name="osbp", bufs=2) as osbp,
            tc.tile_pool(name="tps", bufs=3, space="PSUM") as tpsp,
            tc.tile_pool(name="dps", bufs=2, space="PSUM") as dpsp,
            tc.tile_pool(name="sps", bufs=1, space="PSUM") as spsp,
            tc.tile_pool(name="ops", bufs=1, space="PSUM") as opsp,
            tc.tile_pool(name="wps", bufs=1, space="PSUM") as wpsp,
        ):
            # Prefetch the full x shard: one casting DMA (f32 -> bf16) per
            # 256-row group on the gpsimd software-DGE queue.
            xin_l = []
            off = 0
            for w in CHUNKS:
                xin = xinp.tile([P, w // P, D_IN], BF16, tag="xin")
                for h in range(w // 256):
                    src = x_d[off + h * 256:off + (h + 1) * 256, :]
                    nc.gpsimd.dma_start(
                        xin[:, 2 * h:2 * h + 2, :],
                        src.rearrange("(s p) f -> p s f", p=P))
                xin_l.append(xin)
                off += w

            # Constants on the scalar HW queue. f32r tiles must be
            # *produced* as f32r for the BIR verifier; the ACT bias slice
            # is bitcast back (same bits).
            pkb = constp.tile([P, PKBW], BF16)
            nc.scalar.dma_start(pkb[:], pkb_d[:])
            ident = pkb[:, 0:128]
            wdT = [pkb[:, 128 + 64 * k:128 + 64 * (k + 1)] for k in range(KC)]
            pkf = constp.tile([P, PKFW], F32R)
            nc.scalar.dma_start(pkf[:], pkf_d[:])
            aT = pkf[0:64, 0:63]
            biasN = pkf[0:64, 63:64].bitcast(F32)
            leafG = pkf[0:63, 64:192]

            # PE warm-up: dummy fp32 matmuls on a zeroed scratch tile keep
            # the PE busy (HAM clock-gate ramp) until the first x group
            # lands.
            warm = constp.tile([P, P], F32)
            nc.vector.memset(warm[:], 0.0)
            wps = wpsp.tile([P, 512], F32, tag="wps")
            for w in range(N_WARM):
                nc.tensor.matmul(
                    wps[:, (w % 4) * P:(w % 4 + 1) * P], warm[:], warm[:],
                    start=True, stop=True)

            state = {}

            def emit_head(c):
                # One 256-row half at a time: 8 bf16 transposes into one
                # PSUM bank -> one 16-bit DVE copy -> then mm1 over the
                # whole chunk.
                w = CHUNKS[c]
                xin = xin_l[c]
                xT = xtpp.tile([P, KC, w], BF16, tag="xT")
                dps = dpsp.tile([64, w], F32, tag="dps")
                for h in range(w // 256):
                    tps = tpsp.tile([P, KC, 256], BF16, tag="tps")
                    for k in range(KC):
                        for s2 in range(2):
                            s = 2 * h + s2
                            nc.tensor.matmul(
                                tps[:, k, s2 * P:(s2 + 1) * P],
                                xin[:, s, k * P:(k + 1) * P],
                                ident,
                                is_transpose=True,
                                start=(s2 == 0),
                                stop=(s2 == 1),
                            )
                    nc.vector.tensor_copy(
                        xT[:, :, h * 256:(h + 1) * 256], tps[:])
                for k in range(KC):
                    nc.tensor.matmul(
                        dps[:], wdT[k], xT[:, k, :],
                        start=(k == 0), stop=(k == KC - 1))

                # softplus blocks: t = exp(D2 + bias); sp = ln(t + 1)
                # (pad rows give ln2, nulled by the zero rows of A).
                t = actp.tile([64, w], F32, tag="texp")
                nc.scalar.activation(t[:], dps[:], AF.Exp, bias=biasN)
                sp = spp.tile([64, w], F32R, tag="sp")
                nc.scalar.activation(sp[:], t[:], AF.Ln, bias=1.0)
                state[c] = sp

            def emit_tail(c):
                # S = A @ sp, C = exp(S), out^T = leafG^T @ C, store.
                w = CHUNKS[c]
                off = sum(CHUNKS[:c])
                sp = state.pop(c)
                sps = spsp.tile([N_ALL, w], F32, tag="sps")
                nc.tensor.matmul(sps[:], aT, sp[:], start=True, stop=True)
                cc = ccp.tile([N_ALL, w], F32R, tag="cc")
                nc.scalar.activation(cc[:], sps[:], AF.Exp)
                ops = opsp.tile([P, w], F32, tag="ops")
                nc.tensor.matmul(ops[:], leafG, cc[:], start=True, stop=True)
                osb = osbp.tile([P, w], F32, tag="osb")
                nc.vector.tensor_copy(osb[:], ops[:])
                nc.sync.dma_start(y_d[:, off:off + w], osb[:])

            # Software-pipelined emission: chunk c+1's transposes+mm1 go
            # to the PE queue before chunk c's mmA/mm2 so the PE never
            # stalls waiting on the ACT exp/ln chain.
            emit_head(0)
            for c in range(1, len(CHUNKS)):
                emit_head(c)
                emit_tail(c - 1)
            emit_tail(len(CHUNKS) - 1)
    return nc


_BUILD_CACHE = {}


def _pin_act_tables(nc):
    """Restrict Exp/Ln to the one table set that holds both, so the
    table-load placement pass emits a single ACT_TABLE_LOAD instead of
    thrashing between per-function sets (~2.7us per switch)."""
    from concourse import hw_specs
    tables = hw_specs.get_activation_tables(nc.m.arch)
    both = "natural_log_exp_and_others"
    if both in tables and AF.Exp in tables[both] and AF.Ln in tables[both]:
        for name, fns in tables.items():
            if name != both:
                fns.discard(AF.Exp)
                fns.discard(AF.Ln)


def build():
    if "nc" not in _BUILD_CACHE:
        nc = bacc.Bacc("TRN2", target_bir_lowering=False, debug=False,
                       num_devices=N_CORES)
        _pin_act_tables(nc)
        _emit(nc)
        nc.compile()
        _BUILD_CACHE["nc"] = nc
    return _BUILD_CACHE["nc"]


def host_prep(W, b, gamma, leaf):
    """Fold the tiny tree parameters into two packed constant blocks."""
    import ml_dtypes
    W = np.asarray(W, np.float32)
    b = np.asarray(b, np.float32)
    gamma = np.asarray(gamma, np.float32)
    leaf = np.asarray(leaf, np.float32)

    Wd = W[:, 0, :] - W[:, 1, :]                      # [31, 512]
    bd = b[:, 0] - b[:, 1]                            # [31]
    e = np.exp(gamma - gamma.max(-1, keepdims=True))
    g = e / e.sum(-1, keepdims=True)                  # [31, 2]

    path = np.zeros(N_ALL, np.float64)
    path[0] = 1.0
    for m in range(1, N_ALL):
        par = (m - 1) // 2
        path[m] = path[par] * g[par, 0]
    G = np.array([path[m] * (g[m, 1] if m < N_INT else 1.0)
                  for m in range(N_ALL)])
    leafG = (G[:, None] * leaf.astype(np.float64)).astype(np.float32)

    # A[row, m] = -1 if the edge lives on the root->m path.
    # Edge (node a, child j) -> row a (j=0) or row 32+a (j=1); rows 31/63 pad.
    A = np.zeros((64, N_ALL), np.float32)
    for m in range(N_ALL):
        node = m
        while node:
            par = (node - 1) // 2
            j = node - 2 * par - 1
            A[par if j == 0 else 32 + par, m] = -1.0
            node = par

    # bf16 block: transpose identity | doubled routing weights
    # (cols 0..30 = -Wd^T, cols 32..62 = +Wd^T per 128-feature chunk).
    pkb = np.zeros((P, PKBW), np.float32)
    pkb[:, 0:128] = np.eye(P, dtype=np.float32)
    wdTfull = np.ascontiguousarray(Wd.T)              # [512, 31]
    for k in range(KC):
        blk = wdTfull[k * P:(k + 1) * P]
        pkb[:, 128 + 64 * k + 0:128 + 64 * k + N_INT] = -blk
        pkb[:, 128 + 64 * k + 32:128 + 64 * k + 32 + N_INT] = blk

    # f32r block: aT | bias | leafG.
    pkf = np.zeros((P, PKFW), np.float32)
    pkf[0:64, 0:63] = A
    pkf[0:N_INT, 63] = -bd
    pkf[32:32 + N_INT, 63] = bd
    pkf[0:N_ALL, 64:192] = leafG
    return {"pkb": pkb.astype(ml_dtypes.bfloat16), "pkf": pkf}


def run(x, W, b, gamma, leaf, **spmd_kwargs):
    x = np.asarray(x, np.float32)
    consts = host_prep(W, b, gamma, leaf)
    shards = x.reshape(N_CORES, B_CORE, D_IN)
    in_maps = [dict(consts, x=np.ascontiguousarray(shards[i]))
               for i in range(N_CORES)]
    nc = build()
    res = run_bass_kernel_spmd(nc, in_maps, list(range(N_CORES)), **spmd_kwargs)
    y = np.concatenate(
        [np.ascontiguousarray(res.results[i]["y"].T) for i in range(N_CORES)],
        axis=0)
    return y, res


def kernel(x, W, b, gamma, leaf):
    y, _ = run(x, W, b, gamma, leaf)
    return y
